# revision 4
# baseline (speedup 1.0000x reference)
"""BitLinear (LayerNorm + absmax-quantize + binary-weight matmul) on 8 trn2
cores.

Sharding: data-parallel over tokens. Each core gets T/8 tokens of x and the
full weight matrix; LayerNorm+quantize are computed per-token on the owning
core, so no collectives are needed.

Matmul strategy: fp8 DoubleRow (2 fp8 MACs/cell/cycle, 0.5 cycles/row) with
a hi/lo two-term split of the quantized activations to recover precision:
  xq = hi + lo,  hi = fp8(clip(xq)),  lo = fp8(clip(xq) - hi)
  y  = hi @ w + lo @ w          (w is +-1, exactly representable in fp8e4)
Both terms accumulate into the same PSUM bank. DoubleRow's LDWEIGHTS is
serial with the matmuls (the mode consumes both PE weight buffers, so
there is no background-buffer overlap), so the loop keeps half of w
resident in SBUF and makes each stationary load serve 4 N=512 matmuls
(one 2048-wide output half per token tile).

hi and lo are written by DVE into the even/odd bytes of one bf16-typed
buffer (strided fp8 views via AP.bitcast), so a single 2-byte XBAR
dma-transpose moves both to feature-major layout; the PE then reads the
stationary operand through strided fp8 views of the transposed tile.

Weights are host-packed to w_pack[ki, kt2, j, n] = w[kt2*256 + j*128 + ki, n]
(fp8), matching the DoubleRow moving-operand pair layout [128, 2, N].

DMA queue split: x loads + XBAR transposes ride the ACT HWDGE queue, w
chunks + y stores the SP queue, so phase-1 traffic never queues in front
of the weight chunks the PE is waiting on.
"""

import functools
import sys
from contextlib import ExitStack

sys.path.insert(0, "/opt/trn_rl_repo")

import ml_dtypes
import numpy as np

import concourse.bass as bass
import concourse.mybir as mybir
import concourse.tile as tile
from concourse import bacc
from concourse.bass_utils import run_bass_kernel_spmd

N_CORES = 8
P = 128
QB = 128.0
EP = 0.01
LN_EPS = 1e-5

F32 = mybir.dt.float32
BF16 = mybir.dt.bfloat16
FP8 = mybir.dt.float8e4


def build(T, D, NOUT, s, out_scale, with_ln_affine, jn_half=2048, lookahead=2,
          repeat=1, emit_phase1=True, emit_phase2=True, dve_copy=False):
    """Emit + compile the per-core program.

    T: tokens per core, D: n_in, NOUT: n_out. s = QB/gamma.
    jn_half: output columns per resident w chunk (psum limits: 2 tokens in
    flight need 2*jn_half*4B <= 16KB -> jn_half <= 2048).
    """
    assert T % P == 0 and D % 256 == 0 and NOUT % jn_half == 0
    G = T // P          # token groups
    KT = D // P         # 128-deep contraction tiles (for the transpose view)
    KT2 = D // 256      # 256-deep DoubleRow contraction tiles
    JH = NOUT // jn_half
    NBH = jn_half // 512
    n_bn = (D + 511) // 512
    assert D % n_bn == 0
    assert 2 * NBH <= 8, "PSUM banks exceeded"

    nc = bacc.Bacc("TRN2", target_bir_lowering=False, debug=False)
    x = nc.declare_dram_parameter("x", [T, D], F32, isOutput=False).ap()
    w = nc.declare_dram_parameter("w", [P, KT2, 2, NOUT], FP8,
                                  isOutput=False).ap()
    y = nc.declare_dram_parameter("y", [T, NOUT], F32, isOutput=True).ap()
    if with_ln_affine:
        ln_g = nc.declare_dram_parameter("ln_g", [D], F32, isOutput=False).ap()
        ln_bs = nc.declare_dram_parameter("ln_bs", [D], F32, isOutput=False).ap()

    clip_hi = float(np.float32(QB) - np.float32(EP))
    inv_s2 = float(1.0 / (np.float64(s) * np.float64(s)))
    eps_s2 = float(np.float64(LN_EPS) * inv_s2)

    with tile.TileContext(nc) as tc, ExitStack() as ctx:
        singles = ctx.enter_context(tc.tile_pool(name="singles", bufs=1))
        xin = ctx.enter_context(tc.tile_pool(name="xin", bufs=2))
        xsp = ctx.enter_context(tc.tile_pool(name="xsp", bufs=2))
        vp = ctx.enter_context(tc.tile_pool(name="vp", bufs=1))
        st = ctx.enter_context(tc.tile_pool(name="st", bufs=4))
        vT_pool = ctx.enter_context(tc.tile_pool(name="vT", bufs=G))
        whalf = ctx.enter_context(tc.tile_pool(name="whalf", bufs=1))
        ysb = ctx.enter_context(tc.tile_pool(name="ysb", bufs=2))
        psum = ctx.enter_context(tc.tile_pool(name="psum", bufs=2,
                                              space="PSUM"))

        # eps tile holds eps/s^2 so that 1/sqrt(var/s^2 + eps/s^2) = s*rstd
        eps_t = singles.tile([P, 1], F32)
        nc.vector.memset(eps_t, eps_s2)

        if with_ln_affine:
            g_b = singles.tile([P, D], F32)
            bs_b = singles.tile([P, D], F32)
            for vec, dst in ((ln_g, g_b), (ln_bs, bs_b)):
                bcast = bass.AP(tensor=vec.tensor, offset=vec.offset,
                                ap=[[0, P]] + list(vec.ap))
                nc.sync.dma_start(out=dst, in_=bcast)

        def emit_phase1_group(g, vT):
            vT_g = vT_pool.tile([P, KT, P], BF16, tag="vT", name=f"vT_{g}")
            if not emit_phase1:
                nc.gpsimd.memset(vT_g, 0)
                vT.append(vT_g)
                return
            x_t = xin.tile([P, D], F32)
            nc.scalar.dma_start(out=x_t, in_=x[g * P:(g + 1) * P, :])

            stats = st.tile([P, n_bn, 6], F32)
            xv = x_t.rearrange("p (n b) -> p n b", n=n_bn)
            for sg in range(n_bn):
                nc.vector.bn_stats(out=stats[:, sg, :], in_=xv[:, sg, :])
            mv = st.tile([P, 2], F32)
            nc.vector.bn_aggr(out=mv, in_=stats)

            # srstd = s / sqrt(var + eps) = 1 / sqrt(var/s^2 + eps/s^2)
            srstd = st.tile([P, 1], F32)
            nc.scalar.activation(out=srstd, in_=mv[:, 1:2],
                                 func=mybir.ActivationFunctionType.Sqrt,
                                 bias=eps_t, scale=inv_s2)
            nc.vector.reciprocal(out=srstd, in_=srstd)
            # b = -mu * srstd
            b_t = st.tile([P, 1], F32)
            nc.vector.tensor_scalar(b_t, mv[:, 0:1], srstd, -1.0,
                                    mybir.AluOpType.mult, mybir.AluOpType.mult)
            # xs = x*srstd + b = (x - mu) * rstd * s   (ACT, bf16 out)
            xs = xsp.tile([P, D], BF16)
            nc.scalar.activation(out=xs, in_=x_t,
                                 func=mybir.ActivationFunctionType.Identity,
                                 bias=b_t, scale=srstd)
            if with_ln_affine:
                nc.vector.tensor_tensor(xs, xs, g_b, mybir.AluOpType.mult)
                nc.vector.tensor_tensor(xs, xs, bs_b, mybir.AluOpType.add)
            # clip in bf16 (so the lo term can't "un-clip"), then split
            nc.vector.tensor_scalar(xs, xs, clip_hi, -clip_hi,
                                    mybir.AluOpType.min, mybir.AluOpType.max)
            v16 = vp.tile([P, D], BF16)
            v8 = v16.bitcast(FP8).rearrange("p (d b) -> p d b", b=2)
            # hi = fp8(xs) -> even bytes; lo = fp8(xs - hi) -> odd bytes
            nc.vector.tensor_copy(v8[:, :, 0], xs)
            nc.vector.tensor_tensor(v8[:, :, 1], xs, v8[:, :, 0],
                                    mybir.AluOpType.subtract)
            nc.scalar.dma_start_transpose(vT_g, v16)
            vT.append(vT_g)

        def emit_once():
            vT = []

            def ensure(g):
                while len(vT) <= g:
                    emit_phase1_group(len(vT), vT)

            for g in range(min(lookahead, G)):
                ensure(g)
            if not emit_phase2:
                ensure(G - 1)
                for g in range(G):
                    yo = ysb.tile([P, 8], F32, name="yo_dummy")
                    nc.vector.tensor_copy(yo, vT[g][:, 0, 0:8])
                    nc.sync.dma_start(out=y[g * P:(g + 1) * P, 0:8], in_=yo)
                return

            for jh in range(JH):
                w_h = whalf.tile([P, KT2, 2, jn_half], FP8, tag="wh")
                for kt2 in range(KT2):
                    nc.sync.dma_start(
                        out=w_h[:, kt2],
                        in_=w[:, kt2, :, jh * jn_half:(jh + 1) * jn_half])
                for t in range(G):
                    if jh == 0:
                        ensure(min(t + lookahead, G - 1))
                    v8T = vT[t].bitcast(FP8).rearrange(
                        "p kt (t b) -> p kt t b", b=2)
                    ps = psum.tile([P, NBH, 512], F32, tag="ps")
                    n_mm = 0
                    for kt2 in range(KT2):
                        for hb in range(2):   # hi then lo
                            stat = v8T[:, 2 * kt2:2 * kt2 + 2, :, hb]
                            for nb in range(NBH):
                                nc.tensor.matmul(
                                    ps[:, nb, :], stat,
                                    w_h[:, kt2, :, nb * 512:(nb + 1) * 512],
                                    start=(kt2 == 0 and hb == 0),
                                    stop=(kt2 == KT2 - 1 and hb == 1),
                                    perf_mode=mybir.MatmulPerfMode.DoubleRow)
                            n_mm += 1
                    yo = ysb.tile([P, jn_half], F32)
                    if dve_copy:
                        nc.vector.tensor_scalar_mul(
                            yo, ps.rearrange("p a b -> p (a b)"), out_scale)
                    else:
                        nc.scalar.mul(out=yo,
                                      in_=ps.rearrange("p a b -> p (a b)"),
                                      mul=out_scale)
                    nc.sync.dma_start(
                        out=y[t * P:(t + 1) * P,
                              jh * jn_half:(jh + 1) * jn_half],
                        in_=yo)

        for _ in range(repeat):
            emit_once()

    nc.compile()
    return nc


BEST = dict(jn_half=2048, lookahead=2, dve_copy=False)


def pack_w(w, D, NOUT):
    """w [D, NOUT] (+-1) -> fp8 w_pack[ki, kt2, j, n] = w[kt2*256+j*128+ki, n]."""
    f8 = mybir.dt.np(FP8)
    KT2 = D // 256
    return np.ascontiguousarray(
        np.asarray(w, dtype=np.float32)
        .reshape(KT2, 2, P, NOUT).transpose(2, 0, 1, 3)).astype(f8)


@functools.lru_cache(maxsize=4)
def _built(T, D, NOUT, s, out_scale, with_ln_affine):
    cfg = dict(BEST)
    if with_ln_affine:
        # ln gamma/beta broadcast tiles take 32KB/partition of SBUF; shrink
        # the resident w chunk to stay within budget.
        cfg["jn_half"] = min(cfg["jn_half"], 1024)
    return build(T, D, NOUT, s, out_scale, with_ln_affine, **cfg)


def kernel(x, w, ln_gamma, ln_beta, beta, gamma):
    B, S, D = x.shape
    NOUT = w.shape[1]
    T_full = B * S
    assert T_full % N_CORES == 0
    T = T_full // N_CORES

    gamma32 = np.float32(gamma)
    s = float(np.float32(QB) / gamma32)
    out_scale = float(np.float32(beta) * gamma32 / np.float32(QB))
    with_ln_affine = not (np.all(ln_gamma == 1.0) and np.all(ln_beta == 0.0))

    w_pack = pack_w(w, D, NOUT)

    nc = _built(T, D, NOUT, s, out_scale, with_ln_affine)

    x_flat = np.ascontiguousarray(x.reshape(T_full, D), dtype=np.float32)
    in_maps = []
    for c in range(N_CORES):
        m = {"x": x_flat[c * T:(c + 1) * T], "w": w_pack}
        if with_ln_affine:
            m["ln_g"] = np.asarray(ln_gamma, dtype=np.float32)
            m["ln_bs"] = np.asarray(ln_beta, dtype=np.float32) * np.float32(s)
        in_maps.append(m)

    res = run_bass_kernel_spmd(nc, in_maps, list(range(N_CORES)))
    out = np.concatenate([res.results[c]["y"] for c in range(N_CORES)], axis=0)
    return out.reshape(B, S, NOUT).astype(np.float32)


# revision 9
# speedup vs baseline: 1.0361x; 1.0361x over previous
"""BitLinear (LayerNorm + absmax-quantize + binary-weight matmul) on 8 trn2
cores.

Sharding: data-parallel over tokens. Each core gets T/8 tokens of x and the
full weight matrix; LayerNorm+quantize are computed per-token on the owning
core, so no collectives are needed.

Matmul strategy: fp8 DoubleRow (2 fp8 MACs/cell/cycle, 0.5 cycles/row) with
a hi/lo two-term split of the quantized activations to recover precision:
  xq = hi + lo,  hi = fp8(clip(xq)),  lo = fp8(clip(xq) - hi)
  y  = hi @ w + lo @ w          (w is +-1, exactly representable in fp8e4)
Both terms accumulate into the same PSUM bank. DoubleRow's LDWEIGHTS is
serial with the matmuls (the mode consumes both PE weight buffers, so
there is no background-buffer overlap), so the loop keeps half of w
resident in SBUF and makes each stationary load serve 4 N=512 matmuls
(one 2048-wide output half per token tile).

hi and lo are written by DVE into the even/odd bytes of one bf16-typed
buffer (strided fp8 views via AP.bitcast), so a single 2-byte XBAR
dma-transpose moves both to feature-major layout; the PE then reads the
stationary operand through strided fp8 views of the transposed tile.

Weights are host-packed to w_pack[ki, kt2, j, n] = w[kt2*256 + j*128 + ki, n]
(fp8), matching the DoubleRow moving-operand pair layout [128, 2, N].

DMA queue split: x loads + XBAR transposes ride the ACT HWDGE queue, w
chunks + y stores the SP queue, so phase-1 traffic never queues in front
of the weight chunks the PE is waiting on.
"""

import functools
import sys
from contextlib import ExitStack

sys.path.insert(0, "/opt/trn_rl_repo")

import ml_dtypes
import numpy as np

import concourse.bass as bass
import concourse.mybir as mybir
import concourse.tile as tile
from concourse import bacc
from concourse.bass_utils import run_bass_kernel_spmd

N_CORES = 8
P = 128
QB = 128.0
EP = 0.01
LN_EPS = 1e-5

F32 = mybir.dt.float32
BF16 = mybir.dt.bfloat16
FP8 = mybir.dt.float8e4


def build(T, D, NOUT, s, out_scale, with_ln_affine, jn_half=2048, lookahead=2,
          repeat=1, emit_phase1=True, emit_phase2=True, dve_copy=False,
          act_dma=True, wsub=2, emit_mm=True):
    """Emit + compile the per-core program.

    T: tokens per core, D: n_in, NOUT: n_out. s = QB/gamma.
    jn_half: output columns per resident w chunk (psum limits: 2 tokens in
    flight need 2*jn_half*4B <= 16KB -> jn_half <= 2048).
    wsub: kt2 tiles per w sub-tile; KT2//wsub sub-tiles double-buffer the
    w stream so the next output-half's weights prefetch while the current
    one's tail matmuls run.
    """
    assert T % P == 0 and D % 256 == 0 and NOUT % jn_half == 0
    G = T // P          # token groups
    KT = D // P         # 128-deep contraction tiles (for the transpose view)
    KT2 = D // 256      # 256-deep DoubleRow contraction tiles
    JH = NOUT // jn_half
    NBH = jn_half // 512
    n_bn = (D + 511) // 512
    assert D % n_bn == 0
    assert 2 * NBH <= 8, "PSUM banks exceeded"
    assert KT2 % wsub == 0
    NSUB = KT2 // wsub

    nc = bacc.Bacc("TRN2", target_bir_lowering=False, debug=False)
    x = nc.declare_dram_parameter("x", [T, D], F32, isOutput=False).ap()
    w = nc.declare_dram_parameter("w", [P, KT2, 2, NOUT], FP8,
                                  isOutput=False).ap()
    y = nc.declare_dram_parameter("y", [T, NOUT], F32, isOutput=True).ap()
    if with_ln_affine:
        ln_g = nc.declare_dram_parameter("ln_g", [D], F32, isOutput=False).ap()
        ln_bs = nc.declare_dram_parameter("ln_bs", [D], F32, isOutput=False).ap()

    clip_hi = float(np.float32(QB) - np.float32(EP))
    inv_s2 = float(1.0 / (np.float64(s) * np.float64(s)))
    eps_s2 = float(np.float64(LN_EPS) * inv_s2)

    with tile.TileContext(nc) as tc, ExitStack() as ctx:
        singles = ctx.enter_context(tc.tile_pool(name="singles", bufs=1))
        xin = ctx.enter_context(tc.tile_pool(name="xin", bufs=2))
        xsp = ctx.enter_context(tc.tile_pool(name="xsp", bufs=2))
        vp = ctx.enter_context(tc.tile_pool(name="vp", bufs=1))
        st = ctx.enter_context(tc.tile_pool(name="st", bufs=4))
        vT_pool = ctx.enter_context(tc.tile_pool(name="vT", bufs=G))
        wq = ctx.enter_context(tc.tile_pool(name="wq", bufs=NSUB))
        ysb = ctx.enter_context(tc.tile_pool(name="ysb", bufs=2))
        psum = ctx.enter_context(tc.tile_pool(name="psum", bufs=2,
                                              space="PSUM"))

        # eps tile holds eps/s^2 so that 1/sqrt(var/s^2 + eps/s^2) = s*rstd
        eps_t = singles.tile([P, 1], F32)
        nc.vector.memset(eps_t, eps_s2)

        if with_ln_affine:
            g_b = singles.tile([P, D], F32)
            bs_b = singles.tile([P, D], F32)
            for vec, dst in ((ln_g, g_b), (ln_bs, bs_b)):
                bcast = bass.AP(tensor=vec.tensor, offset=vec.offset,
                                ap=[[0, P]] + list(vec.ap))
                nc.sync.dma_start(out=dst, in_=bcast)

        def emit_phase1_group(g, vT):
            vT_g = vT_pool.tile([P, KT, P], BF16, tag="vT", name=f"vT_{g}")
            if not emit_phase1:
                nc.gpsimd.memset(vT_g, 0)
                vT.append(vT_g)
                return
            x_t = xin.tile([P, D], F32)
            dma_eng = nc.scalar if act_dma else nc.sync
            dma_eng.dma_start(out=x_t, in_=x[g * P:(g + 1) * P, :])

            stats = st.tile([P, n_bn, 6], F32)
            xv = x_t.rearrange("p (n b) -> p n b", n=n_bn)
            for sg in range(n_bn):
                nc.vector.bn_stats(out=stats[:, sg, :], in_=xv[:, sg, :])
            mv = st.tile([P, 2], F32)
            nc.vector.bn_aggr(out=mv, in_=stats)

            # srstd = s / sqrt(var + eps) = 1 / sqrt(var/s^2 + eps/s^2)
            srstd = st.tile([P, 1], F32)
            nc.scalar.activation(out=srstd, in_=mv[:, 1:2],
                                 func=mybir.ActivationFunctionType.Sqrt,
                                 bias=eps_t, scale=inv_s2)
            nc.vector.reciprocal(out=srstd, in_=srstd)
            # b = -mu * srstd
            b_t = st.tile([P, 1], F32)
            nc.vector.tensor_scalar(b_t, mv[:, 0:1], srstd, -1.0,
                                    mybir.AluOpType.mult, mybir.AluOpType.mult)
            # xs = x*srstd + b = (x - mu) * rstd * s   (ACT, bf16 out)
            xs = xsp.tile([P, D], BF16)
            nc.scalar.activation(out=xs, in_=x_t,
                                 func=mybir.ActivationFunctionType.Identity,
                                 bias=b_t, scale=srstd)
            if with_ln_affine:
                nc.vector.tensor_tensor(xs, xs, g_b, mybir.AluOpType.mult)
                nc.vector.tensor_tensor(xs, xs, bs_b, mybir.AluOpType.add)
            # clip in bf16 (so the lo term can't "un-clip"), then split
            nc.vector.tensor_scalar(xs, xs, clip_hi, -clip_hi,
                                    mybir.AluOpType.min, mybir.AluOpType.max)
            v16 = vp.tile([P, D], BF16)
            v8 = v16.bitcast(FP8).rearrange("p (d b) -> p d b", b=2)
            # hi = fp8(xs) -> even bytes; lo = fp8(xs - hi) -> odd bytes
            nc.vector.tensor_copy(v8[:, :, 0], xs)
            nc.vector.tensor_tensor(v8[:, :, 1], xs, v8[:, :, 0],
                                    mybir.AluOpType.subtract)
            dma_eng.dma_start_transpose(vT_g, v16)
            vT.append(vT_g)

        dma_eng = nc.scalar if act_dma else nc.sync

        if not emit_phase2:
            for _ in range(repeat):
                vT = []
                for g in range(G):
                    emit_phase1_group(g, vT)
                for g in range(G):
                    yo = ysb.tile([P, 8], F32, name="yo_dummy")
                    nc.vector.tensor_copy(yo, vT[g][:, 0, 0:8])
                    nc.sync.dma_start(out=y[g * P:(g + 1) * P, 0:8], in_=yo)
        else:
            passes = [(r, jh) for r in range(repeat) for jh in range(JH)]
            wsubs = {}
            vTs = {}

            def emit_wdma(pi):
                if pi >= len(passes) or pi in wsubs:
                    return
                _, jh = passes[pi]
                tiles = []
                for sb in range(NSUB):
                    wt = wq.tile([P, wsub, 2, jn_half], FP8, tag="wq")
                    nc.sync.dma_start(
                        out=wt,
                        in_=w[:, sb * wsub:(sb + 1) * wsub, :,
                              jh * jn_half:(jh + 1) * jn_half])
                    tiles.append(wt)
                wsubs[pi] = tiles

            def ensure(r, g):
                vT = vTs.setdefault(r, [])
                g = min(g, G - 1)
                while len(vT) <= g:
                    emit_phase1_group(len(vT), vT)

            emit_wdma(0)
            for pi, (r, jh) in enumerate(passes):
                w_t = wsubs.pop(pi)
                for t in range(G):
                    if jh == 0:
                        ensure(r, t + lookahead)
                    if t == G - 2:
                        emit_wdma(pi + 1)
                    v8T = vTs[r][t].bitcast(FP8).rearrange(
                        "p kt (t b) -> p kt t b", b=2)
                    ps = psum.tile([P, NBH, 512], F32, tag="ps")
                    if emit_mm:
                        for kt2 in range(KT2):
                            for hb in range(2):   # hi then lo
                                stat = v8T[:, 2 * kt2:2 * kt2 + 2, :, hb]
                                wv = w_t[kt2 // wsub][:, kt2 % wsub]
                                for nb in range(NBH):
                                    nc.tensor.matmul(
                                        ps[:, nb, :], stat,
                                        wv[:, :, nb * 512:(nb + 1) * 512],
                                        start=(kt2 == 0 and hb == 0),
                                        stop=(kt2 == KT2 - 1 and hb == 1),
                                        perf_mode=(
                                            mybir.MatmulPerfMode.DoubleRow))
                    else:
                        nc.gpsimd.memset(ps, 0)
                    yo = ysb.tile([P, jn_half], F32)
                    if dve_copy:
                        nc.vector.tensor_scalar_mul(
                            yo, ps.rearrange("p a b -> p (a b)"), out_scale)
                    else:
                        nc.scalar.mul(out=yo,
                                      in_=ps.rearrange("p a b -> p (a b)"),
                                      mul=out_scale)
                    dma_eng.dma_start(
                        out=y[t * P:(t + 1) * P,
                              jh * jn_half:(jh + 1) * jn_half],
                        in_=yo)
                if jh == JH - 1:
                    vTs.pop(r)

    nc.compile()
    return nc


BEST = dict(jn_half=2048, lookahead=2, dve_copy=False)


def pack_w(w, D, NOUT):
    """w [D, NOUT] (+-1) -> fp8 w_pack[ki, kt2, j, n] = w[kt2*256+j*128+ki, n]."""
    f8 = mybir.dt.np(FP8)
    KT2 = D // 256
    return np.ascontiguousarray(
        np.asarray(w, dtype=np.float32)
        .reshape(KT2, 2, P, NOUT).transpose(2, 0, 1, 3)).astype(f8)


@functools.lru_cache(maxsize=4)
def _built(T, D, NOUT, s, out_scale, with_ln_affine):
    cfg = dict(BEST)
    if with_ln_affine:
        # ln gamma/beta broadcast tiles take 32KB/partition of SBUF; shrink
        # the resident w chunk to stay within budget.
        cfg["jn_half"] = min(cfg["jn_half"], 1024)
    return build(T, D, NOUT, s, out_scale, with_ln_affine, **cfg)


def kernel(x, w, ln_gamma, ln_beta, beta, gamma):
    B, S, D = x.shape
    NOUT = w.shape[1]
    T_full = B * S
    assert T_full % N_CORES == 0
    T = T_full // N_CORES

    gamma32 = np.float32(gamma)
    s = float(np.float32(QB) / gamma32)
    out_scale = float(np.float32(beta) * gamma32 / np.float32(QB))
    with_ln_affine = not (np.all(ln_gamma == 1.0) and np.all(ln_beta == 0.0))

    w_pack = pack_w(w, D, NOUT)

    nc = _built(T, D, NOUT, s, out_scale, with_ln_affine)

    x_flat = np.ascontiguousarray(x.reshape(T_full, D), dtype=np.float32)
    in_maps = []
    for c in range(N_CORES):
        m = {"x": x_flat[c * T:(c + 1) * T], "w": w_pack}
        if with_ln_affine:
            m["ln_g"] = np.asarray(ln_gamma, dtype=np.float32)
            m["ln_bs"] = np.asarray(ln_beta, dtype=np.float32) * np.float32(s)
        in_maps.append(m)

    res = run_bass_kernel_spmd(nc, in_maps, list(range(N_CORES)))
    out = np.concatenate([res.results[c]["y"] for c in range(N_CORES)], axis=0)
    return out.reshape(B, S, NOUT).astype(np.float32)
